# revision 24
# baseline (speedup 1.0000x reference)
"""Expert-parallel MoE SwiGLU kernel for one TRN2 chip (8 NeuronCores).

Problem: out[n] = sum_k w[n,k] * FFN_{idx[n,k]}(x[n]) with E=8 experts,
top-2 routing, H=1024, I=4096, N=2048 tokens.

Strategy: one expert per core. Tokens are routed (gathered) per expert on
the host, each core runs the three bf16 matmuls of its expert's SwiGLU FFN
(silu(x@w1) * (x@w3)) @ w2 over its token batch entirely transposed
(tokens along the PE moving/free dim), and the host scatter-adds the
returned per-expert outputs with the routing weights. Expert token counts
above the per-core capacity (PE moving-dim chunk of 512) spill to a small
host-side f32 pass so the device runs a single full-width chunk.

Schedule notes (from NTFF profile analysis): the kernel is tensor-bound
(768 N=512 bf16 matmuls ~= 166us at 2.4GHz). The startup-critical inputs
(x 1MB + first w13 tile 0.5MB) are split across the three independent DMA
rings (sync + scalar hardware DGE, gpsimd software DGE, ~150GB/s each) so
the first real matmul issues ~11us after kernel start instead of ~21us.
The w13/w2 weight streams alternate between the gpsimd and scalar rings so
each ring only needs ~75GB/s sustained. PE warmup matmuls (HAM clock ramp)
cover the input DMA window.
"""

import sys

for _p in ("/opt/trn_rl_repo", "/opt/pypackages"):
    if _p not in sys.path:
        sys.path.insert(0, _p)

import numpy as np
import ml_dtypes

import concourse.tile as tile
from concourse import bacc, mybir
from concourse.bass_utils import run_bass_kernel_spmd

P = 128
H = 1024
I = 4096
KH = H // P    # 8 contraction subtiles for the first matmuls
KH2 = KH // 2  # kh subtiles per x half-tile
II = I // P    # 32 intermediate subtiles / contraction subtiles for w2
CAP = 512      # per-core token capacity (single PE moving chunk)
# PE warmup matmuls: ramp the HAM clock AND keep the tensor engine busy
# until the startup-critical DMAs are fully resident (~8.2us after the
# first warmup matmul). Overshoot costs 109ns/matmul; undershoot risks a
# HAM re-throttle that halves the early real matmul rate.
N_WARM = 60
W_COLS = 256   # warmup matmul moving width (finer tail granularity)

BF16 = mybir.dt.bfloat16
F32 = mybir.dt.float32


def _build(C):
    """One-expert SwiGLU FFN over C tokens (C <= 512), transposed layout.

    DRAM inputs (per core):
      xg   [P, KH, C]       bf16  x^T: [hp, kh, c] = x[tok c, kh*P+hp]
      w13t [II/2, P, 2, 2, KH, P] bf16, pair-major ii blocks:
           [j, hp, a, 0, kh, m] = w1[kh*P+hp, (2j+a)*P+m]
           [j, hp, a, 1, kh, m] = w3[kh*P+hp, (2j+a)*P+m]
      w2t  [KH, P, II, P]   bf16  [hh, ip, ik, m] = w2[ik*P+ip, hh*P+m]
    Output:
      yt   [KH, P, C]       f32   y^T tiled by output subtile
    """
    assert C <= 512
    nc = bacc.Bacc("TRN2", target_bir_lowering=False, debug=False)
    xg = nc.dram_tensor("xg", [P, KH, C], BF16, kind="ExternalInput")
    w13t = nc.dram_tensor(
        "w13t", [II // 2, P, 2, 2, KH, P], BF16, kind="ExternalInput"
    )
    w2t = nc.dram_tensor("w2t", [KH, P, II, P], BF16, kind="ExternalInput")
    yt = nc.dram_tensor("yt", [KH, P, C], F32, kind="ExternalOutput")

    with tile.TileContext(nc) as tc:
        with (
            tc.tile_pool(name="xp", bufs=1) as xp,
            tc.tile_pool(name="pp", bufs=1) as pp,
            tc.tile_pool(name="wp", bufs=10) as wp,
            tc.tile_pool(name="w2p", bufs=3) as w2p,
            tc.tile_pool(name="gp", bufs=4) as gp,
            tc.tile_pool(name="yp", bufs=3) as yp,
            tc.tile_pool(name="warm", bufs=1) as warm,
            tc.tile_pool(name="psA", bufs=3, space="PSUM") as psA,
            tc.tile_pool(name="psB", bufs=2, space="PSUM") as psB,
        ):
            # Startup-critical loads, spread over the independent DMA rings
            # (~150GB/s each) in the order the rings come up: sync ~8.7us,
            # scalar ~9.9us (after its ACT table load), gpsimd software DGE
            # ~10.6us (ucode desc-gen latency). Each chunk is its own tile so
            # the first matmul chains only wait for the data they read
            # (whole-tile deps defeat chunked DMAs into one tile).
            w10 = wp.tile([P, KH, P], BF16, tag="w1h", bufs=1)
            nc.scalar.dma_start(w10[:], w13t[0][:, 0, 0])   # ii=0 w1 half
            xch = []
            for j in range(4):
                xc = xp.tile([P, 2, C], BF16, tag=f"x{j}", bufs=1)
                eng = nc.sync if j != 2 else nc.scalar
                eng.dma_start(xc[:], xg[:, 2 * j : 2 * j + 2, :])
                xch.append(xc)
            # w3 half lands last; the pg (w1) chain is then schedulable
            # strictly before the pu chain.
            w30 = wp.tile([P, KH, P], BF16, tag="w3h", bufs=1)
            nc.scalar.dma_start(w30[:], w13t[0][:, 0, 1])   # ii=0 w3 half

            def xh(kh):
                return xch[kh // 2][:, kh % 2, :]

            # PE warmup: ramp the tensor engine to high-activity clock while
            # the input DMAs are in flight. Reads a zeroed tile, result is
            # never consumed.
            wtile = warm.tile([P, W_COLS], BF16)
            nc.vector.memset(wtile[:], 0.0)
            # Shares the Phase B psum pool (tag "py"): warmup is long done
            # before Phase B allocates its first chain psum.
            wps = psB.tile([P, W_COLS], F32, tag="py")
            for i in range(N_WARM):
                nc.tensor.matmul(
                    wps, wtile[:, :P], wtile[:], start=(i == 0),
                    stop=(i == N_WARM - 1),
                )

            psb = pp.tile([P, II, C], BF16)

            # Phase A: h1 = silu(x@w1), h3 = x@w3, p = h1*h3 (all transposed)
            # w13 stream: ii=1..3 as single tiles on the gpsimd ring (just-in-
            # time at startup), ii>=4 as pair tiles (one DMA per two ii —
            # fewer triggers and semaphores). The scalar ring, free once the
            # startup x chunks land, carries most pairs; gpsimd the rest.
            wpair = None
            for ii in range(II):
                j, a = divmod(ii, 2)
                if ii == 0:
                    wsel = lambda half, kh: (w10 if half == 0 else w30)[:, kh, :]
                elif ii in (1, 2, 3):
                    wsb = wp.tile([P, 2, KH, P], BF16, tag="w13", bufs=3)
                    # ii=1 rides the scalar ring (queued after the startup
                    # x chunks); the gpsimd software-DGE ring starts too
                    # late and too jittery to deliver it in time.
                    eng = nc.scalar if ii == 1 else nc.gpsimd
                    eng.dma_start(wsb[:], w13t[j][:, a])
                    wsel = lambda half, kh, t=wsb: t[:, half, kh, :]
                elif a == 0:
                    wpair = wp.tile([P, 2, 2, KH, P], BF16, tag="w13p", bufs=5)
                    eng = nc.gpsimd if j % 3 == 1 else nc.scalar
                    eng.dma_start(wpair[:], w13t[j])
                    wsel = lambda half, kh, t=wpair: t[:, 0, half, kh, :]
                else:
                    wsel = lambda half, kh, t=wpair: t[:, 1, half, kh, :]
                pg = psA.tile([P, C], F32, tag="pg")
                pu = psA.tile([P, C], F32, tag="pu")
                for kh in range(KH):
                    nc.tensor.matmul(
                        pg,
                        wsel(0, kh),
                        xh(kh),
                        start=(kh == 0),
                        stop=(kh == KH - 1),
                    )
                for kh in range(KH):
                    nc.tensor.matmul(
                        pu,
                        wsel(1, kh),
                        xh(kh),
                        start=(kh == 0),
                        stop=(kh == KH - 1),
                    )
                gs = gp.tile([P, C], BF16, tag="g")
                nc.scalar.activation(gs, pg, mybir.ActivationFunctionType.Silu)
                nc.vector.tensor_tensor(
                    psb[:, ii, :], gs, pu, mybir.AluOpType.mult
                )

            # Phase B: y = p @ w2 (transposed: yT = w2T-contraction over I).
            # The last hh is split column-wise so its first half's copy+DMA
            # overlaps the second half's matmuls (shorter kernel tail).
            for hh in range(KH):
                w2sb = w2p.tile([P, II, P], BF16, tag="w2")
                # hh=0,1 on scalar (its w13 stream drains first at the A->B
                # transition; the gpsimd ring is still backlogged), then
                # alternate.
                eng = nc.scalar if (hh < 2 or hh % 2 == 1) else nc.gpsimd
                eng.dma_start(w2sb[:], w2t[hh])
                halves = [(0, C)] if hh < KH - 1 else [
                    (0, C // 2), (C // 2, C - C // 2),
                ]
                for hi, (c0, cc) in enumerate(halves):
                    py = psB.tile([P, cc], F32, tag="py")
                    for ik in range(II):
                        nc.tensor.matmul(
                            py,
                            w2sb[:, ik, :],
                            psb[:, ik, c0 : c0 + cc],
                            start=(ik == 0),
                            stop=(ik == II - 1),
                        )
                    # DVE copies keep the COPY activation table off the
                    # scalar queue (its ACT_TABLE_LOAD would delay the scalar
                    # DMA ring's startup-critical triggers by ~1.3us).
                    if hh < KH - 1 or hi == 0:
                        ysb = yp.tile([P, cc], F32, tag="y")
                        nc.vector.tensor_copy(ysb, py)
                        nc.sync.dma_start(yt[hh, :, c0 : c0 + cc], ysb[:])
                    else:
                        # Final chunk: two pipelined copy+store pieces on the
                        # two hardware DGE rings to shorten the kernel tail.
                        h2 = cc // 2
                        ya = yp.tile([P, h2], F32, tag="y")
                        nc.vector.tensor_copy(ya, py[:, :h2])
                        nc.sync.dma_start(yt[hh, :, c0 : c0 + h2], ya[:])
                        yb = yp.tile([P, cc - h2], F32, tag="y")
                        nc.vector.tensor_copy(yb, py[:, h2:])
                        nc.scalar.dma_start(
                            yt[hh, :, c0 + h2 : c0 + cc], yb[:]
                        )

    nc.compile()
    return nc


_PROGRAM_CACHE = {}


def _host_swiglu(x, w1e, w2e, w3e):
    g = x @ w1e
    u = x @ w3e
    g = g / (1.0 + np.exp(-g))
    return (g * u) @ w2e


def kernel(x, expert_indices, expert_weights, w1, w2, w3):
    x = np.asarray(x, dtype=np.float32)
    idx = np.asarray(expert_indices)
    wts = np.asarray(expert_weights, dtype=np.float32)
    w1 = np.asarray(w1, dtype=np.float32)
    w2 = np.asarray(w2, dtype=np.float32)
    w3 = np.asarray(w3, dtype=np.float32)
    N = x.shape[0]
    E = w1.shape[0]
    bf16 = ml_dtypes.bfloat16

    # host-side routing: token list (with multiplicity) per expert; tokens
    # beyond CAP spill to the host f32 path (tiny tail, keeps device at one
    # full-width PE chunk)
    toks, tokw, spill_toks, spill_w = [], [], [], []
    for e in range(E):
        rows, cols = np.nonzero(idx == e)
        w_e = wts[rows, cols]
        toks.append(rows[:CAP])
        tokw.append(w_e[:CAP])
        spill_toks.append(rows[CAP:])
        spill_w.append(w_e[CAP:])
    C = max(16, max(len(t) for t in toks))
    C = ((C + 15) // 16) * 16

    if C not in _PROGRAM_CACHE:
        _PROGRAM_CACHE[C] = _build(C)
    nc = _PROGRAM_CACHE[C]

    in_maps = []
    for e in range(E):
        xt = np.zeros((C, H), dtype=np.float32)
        if len(toks[e]):
            xt[: len(toks[e])] = x[toks[e]]
        # [C, H] -> [hp, kh, c]
        xge = xt.T.reshape(KH, P, C).transpose(1, 0, 2)
        # w1/w3 [H, I] -> [ii, hp, {w1,w3}, kh, m] -> pair-major
        # [ii/2, hp, ii%2, {w1,w3}, kh, m]
        w13 = np.stack(
            [
                w1[e].reshape(KH, P, II, P).transpose(2, 1, 0, 3),
                w3[e].reshape(KH, P, II, P).transpose(2, 1, 0, 3),
            ],
            axis=2,
        )  # [II, P, 2, KH, P]
        w13 = w13.reshape(II // 2, 2, P, 2, KH, P).swapaxes(1, 2)
        in_maps.append(
            {
                "xg": np.ascontiguousarray(xge.astype(bf16)),
                "w13t": np.ascontiguousarray(w13.astype(bf16)),
                "w2t": np.ascontiguousarray(
                    w2[e].reshape(II, P, KH, P).transpose(2, 1, 0, 3).astype(bf16)
                ),
            }
        )

    res = run_bass_kernel_spmd(nc, in_maps, core_ids=list(range(E)))

    out = np.zeros((N, H), dtype=np.float32)
    for e in range(E):
        cnt = len(toks[e])
        if cnt:
            y = res.results[e]["yt"].reshape(H, C).T[:cnt]
            np.add.at(out, toks[e], y * tokw[e][:, None])
        if len(spill_toks[e]):
            ys = _host_swiglu(x[spill_toks[e]], w1[e], w2[e], w3[e])
            np.add.at(out, spill_toks[e], ys * spill_w[e][:, None])
    return out


# revision 28
# speedup vs baseline: 1.2295x; 1.2295x over previous
"""Expert-parallel MoE SwiGLU kernel for one TRN2 chip (8 NeuronCores).

Problem: out[n] = sum_k w[n,k] * FFN_{idx[n,k]}(x[n]) with E=8 experts,
top-2 routing, H=1024, I=4096, N=2048 tokens.

Strategy: one expert per core. Tokens are routed (gathered) per expert on
the host, each core runs the three bf16 matmuls of its expert's SwiGLU FFN
(silu(x@w1) * (x@w3)) @ w2 over its token batch entirely transposed
(tokens along the PE moving/free dim), and the host scatter-adds the
returned per-expert outputs with the routing weights. Expert token counts
above the per-core capacity (PE moving-dim chunk of 512) spill to a small
host-side f32 pass so the device runs a single full-width chunk.

Schedule notes (from NTFF profile analysis): the kernel is tensor-bound
(768 N=512 bf16 matmuls ~= 166us at 2.4GHz). The startup-critical inputs
(x 1MB + first w13 tile 0.5MB) are split across the three independent DMA
rings (sync + scalar hardware DGE, gpsimd software DGE, ~150GB/s each) so
the first real matmul issues ~11us after kernel start instead of ~21us.
The w13/w2 weight streams alternate between the gpsimd and scalar rings so
each ring only needs ~75GB/s sustained. PE warmup matmuls (HAM clock ramp)
cover the input DMA window.
"""

import sys

for _p in ("/opt/trn_rl_repo", "/opt/pypackages"):
    if _p not in sys.path:
        sys.path.insert(0, _p)

import numpy as np
import ml_dtypes

import concourse.tile as tile
from concourse import bacc, mybir
from concourse.bass_utils import run_bass_kernel_spmd

P = 128
H = 1024
I = 4096
KH = H // P    # 8 contraction subtiles for the first matmuls
KH2 = KH // 2  # kh subtiles per x half-tile
II = I // P    # 32 intermediate subtiles / contraction subtiles for w2
CAP = 512      # per-core token capacity (single PE moving chunk)
# PE warmup matmuls: ramp the HAM clock AND keep the tensor engine busy
# until the startup-critical DMAs are fully resident (~6us after the
# first warmup matmul). Overshoot costs 109ns/matmul; undershoot risks a
# HAM re-throttle that halves the early real matmul rate.
N_WARM = 40
W_COLS = 256   # warmup matmul moving width (finer tail granularity)

BF16 = mybir.dt.bfloat16
F32 = mybir.dt.float32


def _build(C):
    """One-expert SwiGLU FFN over C tokens (C <= 512), transposed layout.

    DRAM inputs (per core):
      xg   [P, KH, C]       bf16  x^T: [hp, kh, c] = x[tok c, kh*P+hp]
      w13t [II/2, P, 2, 2, KH, P] bf16, pair-major ii blocks:
           [j, hp, a, 0, kh, m] = w1[kh*P+hp, (2j+a)*P+m]
           [j, hp, a, 1, kh, m] = w3[kh*P+hp, (2j+a)*P+m]
      w2t  [KH, P, II, P]   bf16  [hh, ip, ik, m] = w2[ik*P+ip, hh*P+m]
    Output:
      yt   [KH, P, C]       f32   y^T tiled by output subtile
    """
    assert C <= 512
    nc = bacc.Bacc("TRN2", target_bir_lowering=False, debug=False)
    xg = nc.dram_tensor("xg", [P, KH, C], BF16, kind="ExternalInput")
    w13t = nc.dram_tensor(
        "w13t", [II // 2, P, 2, 2, KH, P], BF16, kind="ExternalInput"
    )
    w2t = nc.dram_tensor("w2t", [KH, P, II, P], BF16, kind="ExternalInput")
    yt = nc.dram_tensor("yt", [KH, P, C], F32, kind="ExternalOutput")

    with tile.TileContext(nc) as tc:
        with (
            tc.tile_pool(name="xp", bufs=1) as xp,
            tc.tile_pool(name="pp", bufs=1) as pp,
            tc.tile_pool(name="wp", bufs=10) as wp,
            tc.tile_pool(name="w2p", bufs=3) as w2p,
            tc.tile_pool(name="gp", bufs=4) as gp,
            tc.tile_pool(name="yp", bufs=3) as yp,
            tc.tile_pool(name="warm", bufs=1) as warm,
            tc.tile_pool(name="psA", bufs=3, space="PSUM") as psA,
            tc.tile_pool(name="psB", bufs=2, space="PSUM") as psB,
        ):
            # Startup-critical loads. DMA ring throughput scales with the
            # per-partition line size (2KB lines ~80GB/s, 4KB ~150GB/s, 8KB
            # ~200+GB/s), so keep transfers WHOLE: x (8KB lines) rides the
            # sync ring (up ~8.7us), w13[0] (4KB lines) the scalar ring
            # (~9.9us, after its ACT table load). Both land ~14us.
            xsb = xp.tile([P, KH, C], BF16)
            nc.sync.dma_start(xsb[:], xg[:])
            w13sb0 = wp.tile([P, 2, KH, P], BF16, tag="w13", bufs=4)
            nc.scalar.dma_start(w13sb0[:], w13t[0][:, 0])

            def xh(kh):
                return xsb[:, kh, :]

            # PE warmup: ramp the tensor engine to high-activity clock while
            # the input DMAs are in flight. Reads a zeroed tile, result is
            # never consumed.
            wtile = warm.tile([P, W_COLS], BF16)
            nc.vector.memset(wtile[:], 0.0)
            # Shares the Phase B psum pool (tag "py"): warmup is long done
            # before Phase B allocates its first chain psum.
            wps = psB.tile([P, W_COLS], F32, tag="py")
            for i in range(N_WARM):
                nc.tensor.matmul(
                    wps, wtile[:, :P], wtile[:], start=(i == 0),
                    stop=(i == N_WARM - 1),
                )

            psb = pp.tile([P, II, C], BF16)

            # Phase A: h1 = silu(x@w1), h3 = x@w3, p = h1*h3 (all transposed)
            # w13 stream: ii=1..3 as single tiles on the gpsimd ring (just-in-
            # time at startup), ii>=4 as pair tiles (one DMA per two ii —
            # fewer triggers and semaphores). The scalar ring, free once the
            # startup x chunks land, carries most pairs; gpsimd the rest.
            wpair = None
            for ii in range(II):
                j, a = divmod(ii, 2)
                if ii == 0:
                    wsel = lambda half, kh: w13sb0[:, half, kh, :]
                elif ii in (1, 2, 3):
                    # Just-in-time singles on the gpsimd software-DGE ring
                    # (its first data lands ~15us, one tile per ~3.4us).
                    wsb = wp.tile([P, 2, KH, P], BF16, tag="w13", bufs=4)
                    nc.gpsimd.dma_start(wsb[:], w13t[j][:, a])
                    wsel = lambda half, kh, t=wsb: t[:, half, kh, :]
                elif a == 0:
                    wpair = wp.tile([P, 2, 2, KH, P], BF16, tag="w13p", bufs=5)
                    # j=2,3 on scalar (gpsimd is busy with the JIT singles
                    # until ~22us); afterwards alternate rings.
                    eng = nc.scalar if (j < 4 or j % 2 == 1) else nc.gpsimd
                    eng.dma_start(wpair[:], w13t[j])
                    wsel = lambda half, kh, t=wpair: t[:, 0, half, kh, :]
                else:
                    wsel = lambda half, kh, t=wpair: t[:, 1, half, kh, :]
                pg = psA.tile([P, C], F32, tag="pg")
                pu = psA.tile([P, C], F32, tag="pu")
                for kh in range(KH):
                    nc.tensor.matmul(
                        pg,
                        wsel(0, kh),
                        xh(kh),
                        start=(kh == 0),
                        stop=(kh == KH - 1),
                    )
                for kh in range(KH):
                    nc.tensor.matmul(
                        pu,
                        wsel(1, kh),
                        xh(kh),
                        start=(kh == 0),
                        stop=(kh == KH - 1),
                    )
                gs = gp.tile([P, C], BF16, tag="g")
                nc.scalar.activation(gs, pg, mybir.ActivationFunctionType.Silu)
                nc.vector.tensor_tensor(
                    psb[:, ii, :], gs, pu, mybir.AluOpType.mult
                )

            # Phase B: y = p @ w2 (transposed: yT = w2T-contraction over I).
            # The last hh is split column-wise so its first half's copy+DMA
            # overlaps the second half's matmuls (shorter kernel tail).
            for hh in range(KH):
                w2sb = w2p.tile([P, II, P], BF16, tag="w2")
                # hh=0,1 on scalar (its w13 stream drains first at the A->B
                # transition; the gpsimd ring is still backlogged), then
                # alternate.
                eng = nc.scalar if (hh < 2 or hh % 2 == 1) else nc.gpsimd
                eng.dma_start(w2sb[:], w2t[hh])
                halves = [(0, C)] if hh < KH - 1 else [
                    (0, C // 2), (C // 2, C - C // 2),
                ]
                for hi, (c0, cc) in enumerate(halves):
                    py = psB.tile([P, cc], F32, tag="py")
                    for ik in range(II):
                        nc.tensor.matmul(
                            py,
                            w2sb[:, ik, :],
                            psb[:, ik, c0 : c0 + cc],
                            start=(ik == 0),
                            stop=(ik == II - 1),
                        )
                    # DVE copies keep the COPY activation table off the
                    # scalar queue (its ACT_TABLE_LOAD would delay the scalar
                    # DMA ring's startup-critical triggers by ~1.3us).
                    if hh < KH - 1 or hi == 0:
                        ysb = yp.tile([P, cc], F32, tag="y")
                        nc.vector.tensor_copy(ysb, py)
                        nc.sync.dma_start(yt[hh, :, c0 : c0 + cc], ysb[:])
                    else:
                        # Final chunk: two pipelined copy+store pieces on the
                        # two hardware DGE rings to shorten the kernel tail.
                        h2 = cc // 2
                        ya = yp.tile([P, h2], F32, tag="y")
                        nc.vector.tensor_copy(ya, py[:, :h2])
                        nc.sync.dma_start(yt[hh, :, c0 : c0 + h2], ya[:])
                        yb = yp.tile([P, cc - h2], F32, tag="y")
                        nc.vector.tensor_copy(yb, py[:, h2:])
                        nc.scalar.dma_start(
                            yt[hh, :, c0 + h2 : c0 + cc], yb[:]
                        )

    nc.compile()
    return nc


_PROGRAM_CACHE = {}


def _host_swiglu(x, w1e, w2e, w3e):
    g = x @ w1e
    u = x @ w3e
    g = g / (1.0 + np.exp(-g))
    return (g * u) @ w2e


def kernel(x, expert_indices, expert_weights, w1, w2, w3):
    x = np.asarray(x, dtype=np.float32)
    idx = np.asarray(expert_indices)
    wts = np.asarray(expert_weights, dtype=np.float32)
    w1 = np.asarray(w1, dtype=np.float32)
    w2 = np.asarray(w2, dtype=np.float32)
    w3 = np.asarray(w3, dtype=np.float32)
    N = x.shape[0]
    E = w1.shape[0]
    bf16 = ml_dtypes.bfloat16

    # host-side routing: token list (with multiplicity) per expert; tokens
    # beyond CAP spill to the host f32 path (tiny tail, keeps device at one
    # full-width PE chunk)
    toks, tokw, spill_toks, spill_w = [], [], [], []
    for e in range(E):
        rows, cols = np.nonzero(idx == e)
        w_e = wts[rows, cols]
        toks.append(rows[:CAP])
        tokw.append(w_e[:CAP])
        spill_toks.append(rows[CAP:])
        spill_w.append(w_e[CAP:])
    C = max(16, max(len(t) for t in toks))
    C = ((C + 15) // 16) * 16

    if C not in _PROGRAM_CACHE:
        _PROGRAM_CACHE[C] = _build(C)
    nc = _PROGRAM_CACHE[C]

    in_maps = []
    for e in range(E):
        xt = np.zeros((C, H), dtype=np.float32)
        if len(toks[e]):
            xt[: len(toks[e])] = x[toks[e]]
        # [C, H] -> [hp, kh, c]
        xge = xt.T.reshape(KH, P, C).transpose(1, 0, 2)
        # w1/w3 [H, I] -> [ii, hp, {w1,w3}, kh, m] -> pair-major
        # [ii/2, hp, ii%2, {w1,w3}, kh, m]
        w13 = np.stack(
            [
                w1[e].reshape(KH, P, II, P).transpose(2, 1, 0, 3),
                w3[e].reshape(KH, P, II, P).transpose(2, 1, 0, 3),
            ],
            axis=2,
        )  # [II, P, 2, KH, P]
        w13 = w13.reshape(II // 2, 2, P, 2, KH, P).swapaxes(1, 2)
        in_maps.append(
            {
                "xg": np.ascontiguousarray(xge.astype(bf16)),
                "w13t": np.ascontiguousarray(w13.astype(bf16)),
                "w2t": np.ascontiguousarray(
                    w2[e].reshape(II, P, KH, P).transpose(2, 1, 0, 3).astype(bf16)
                ),
            }
        )

    res = run_bass_kernel_spmd(nc, in_maps, core_ids=list(range(E)))

    out = np.zeros((N, H), dtype=np.float32)
    for e in range(E):
        cnt = len(toks[e])
        if cnt:
            y = res.results[e]["yt"].reshape(H, C).T[:cnt]
            np.add.at(out, toks[e], y * tokw[e][:, None])
        if len(spill_toks[e]):
            ys = _host_swiglu(x[spill_toks[e]], w1[e], w2[e], w3[e])
            np.add.at(out, spill_toks[e], ys * spill_w[e][:, None])
    return out
